# Initial kernel scaffold
#
"""AttentionFlow Trainium2 Bass kernel.

Math (per batch):
  d = 256; w = [w_c | w_q | w_m]
  s_c[t] = C[t,:] @ w_c ; s_q[j] = Q[j,:] @ w_q ; s_m[t,j] = sum_d C[t,d] w_m[d] Q[j,d]
  sim = s_c[:,None] + s_q[None,:] + s_m + b
  attn = softmax_j(sim)                  (s_c[t] and b cancel -> softmax_j(g), g = s_m + s_q)
  AQ = attn @ Q
  beta = softmax_t(max_j sim)            (b cancels; numerator n[t] = exp(max_j g[t,:]) * exp(s_c[t]))
  AC = beta @ C
  out = concat([C, AQ, C*AQ, C*AC], axis=-1)

Sharding: data-parallel over batch B=32 across 8 NeuronCores (4 batches/core).

On-chip layout per batch (core-local):
  C    [128(t), 8, 256]      natural t-tiles
  ctT  [128(d), 2, 8, 128]   C^T via PE transposes (d-tile k, t-tile i)
  g^T  [128(j), t]           PE matmul, contraction over d, 512-wide chunks
  E^T  = exp(g^T + s_q)      one ACT pass/chunk, per-partition bias = s_q column
  U    [t,257] = E @ [Q|1]   PE, lhsT = E^T slice (no transpose of attn needed)
  AQ   = U[:,0:256] * 1/U[:,256]
  n    = rowmax(E) * exp(s_c)  (rowmax over partitions via gpsimd
                                partition_all_reduce(max) on E^T; s_c row
                                via 1-col-LDW fp32r matmuls + P=1 transposes)
  AC   = (n^T @ C) / (n^T @ 1); broadcast via gpsimd partition_broadcast

Performance structure:
  - The heavy matmuls (sim, AQ, AC) run in float32r (1 cycle/row vs 4 for
    float32 at moving free-dim >= 256; N=1 moving is ISA-illegal for fp32r).
    The BIR verifier requires fp32r matmul operands to be *written rounded*
    by a compute instruction (DMA-written buffers won't do), so ACT/DVE
    copies produce rounded qwT/ctT/ET/Qr/Cr/n.
  - The beta tail of batch b (n transposes, AC, broadcast, out4, output
    DMAs) is software-pipelined into batch b+1's body so the PE stream
    never stalls on the GpSimd reduce chain.
  - Outputs are staged in per-batch tiles and written with a few large
    DMAs (SP descriptor dispatch is ~0.7us per dma_start; per-tile DMAs
    were the original bottleneck). out[:, :, 0:256] is DMA'd straight
    from the C tile.
"""

import numpy as np

import concourse.bass as bass
import concourse.mybir as mybir
import concourse.tile as tile
from concourse import bacc
from concourse import bass_isa
from concourse.bass_utils import run_bass_kernel_spmd
from concourse.masks import make_identity

F32 = mybir.dt.float32
F32R = mybir.dt.float32r
AF = mybir.ActivationFunctionType
ALU = mybir.AluOpType
AX = mybir.AxisListType

B, T, J, D = 32, 1024, 128, 256
NCORES = 8
BPC = B // NCORES      # batches per core
NT = T // 128          # t-tiles per batch
ND = D // 128          # d-tiles

# float32r for the two big matmuls; set False for an all-fp32 fallback.
USE_F32R = True


def _rr(ap):
    """float32r view of an f32 AP (for rounded producers + matmul operands)."""
    return ap.bitcast(F32R) if USE_F32R else ap


def _bcast_row(ap_1d, nparts):
    """DRAM AP [n] -> [nparts, n] with partition stride 0 (DMA broadcast)."""
    return bass.AP(
        tensor=ap_1d.tensor, offset=ap_1d.offset, ap=[[0, nparts]] + list(ap_1d.ap)
    )


def build_nc(use_f32r=None):
    global USE_F32R
    if use_f32r is not None:
        USE_F32R = use_f32r
    nc = bacc.Bacc()
    ctx_in = nc.declare_dram_parameter("context", [BPC, T, D], F32, isOutput=False)
    qry_in = nc.declare_dram_parameter("query", [BPC, J, D], F32, isOutput=False)
    w_in = nc.declare_dram_parameter("w", [3 * D], F32, isOutput=False)
    out_ext = nc.declare_dram_parameter("out", [BPC, T, 4 * D], F32, isOutput=True)

    with tile.TileContext(nc) as tc:
        _body(tc, ctx_in, qry_in, w_in, out_ext)
    nc.finalize()
    return nc


def _body(tc, ctx_in, qry_in, w_in, out_ext):
    nc = tc.nc
    from contextlib import ExitStack

    with ExitStack() as ctx:
        consts = ctx.enter_context(tc.tile_pool(name="consts", bufs=1))
        big = ctx.enter_context(tc.tile_pool(name="big", bufs=2))
        work = ctx.enter_context(tc.tile_pool(name="work", bufs=4))
        batch = ctx.enter_context(tc.tile_pool(name="batch", bufs=2))
        # PSUM budget (8 banks): g 2 + ct 2 + u 2 + s 1 + ac 1 = 8
        ps_g = ctx.enter_context(tc.tile_pool(name="ps_g", bufs=1, space="PSUM"))
        ps_ct = ctx.enter_context(tc.tile_pool(name="ps_ct", bufs=2, space="PSUM"))
        ps_u = ctx.enter_context(tc.tile_pool(name="ps_u", bufs=3, space="PSUM"))
        ps_s = ctx.enter_context(tc.tile_pool(name="ps_s", bufs=1, space="PSUM"))
        ps_ac = ctx.enter_context(tc.tile_pool(name="ps_ac", bufs=1, space="PSUM"))

        # --- constants (identity first: it gates the first PE transposes) ---
        ident = consts.tile([128, 128], F32)
        make_identity(nc, ident)
        ones_col = consts.tile([128, 1], F32)
        nc.vector.memset(ones_col, 1.0)
        ones_row = consts.tile([1, 128], F32)
        nc.vector.memset(ones_row, 1.0)
        ones2 = consts.tile([128, 2], F32)
        nc.vector.memset(ones2, 1.0)
        ones2_r = consts.tile([128, 2], F32)
        nc.scalar.copy(_rr(ones2_r), ones2)
        ones_row_r = consts.tile([1, 128], F32)
        nc.scalar.copy(_rr(ones_row_r), ones_row)

        # w_c / w_m as per-partition columns (two d-tiles each)
        wc_raw = consts.tile([128, ND], F32)
        wm_cols = consts.tile([128, ND], F32)
        for k in range(ND):
            nc.gpsimd.dma_start(
                out=wc_raw[:, k : k + 1],
                in_=w_in[k * 128 : (k + 1) * 128].rearrange("(p o) -> p o", o=1),
            )
            nc.gpsimd.dma_start(
                out=wm_cols[:, k : k + 1],
                in_=w_in[2 * D + k * 128 : 2 * D + (k + 1) * 128].rearrange(
                    "(p o) -> p o", o=1
                ),
            )
        # rounded copy so the s_c row matmuls can run in fp32r
        wc_cols = consts.tile([128, ND], F32)
        nc.scalar.copy(_rr(wc_cols), wc_raw)
        # w_q broadcast to all partitions (for s_q = rowsum(Q * w_q))
        wq_b = consts.tile([128, D], F32)
        nc.gpsimd.dma_start(out=wq_b, in_=_bcast_row(w_in[D : 2 * D], 128))

        def beta_tail(S):
            """Finish batch S (the beta path + o4 + output DMAs). Runs inside
            the NEXT batch body so the PE stream never stalls on it."""
            b, C, Cr, n_rows, aqo3, o4_all = S
            n_ps = ps_s.tile([128, NT], F32, tag="s")
            for i in range(NT):
                nr = n_rows[i // 4]
                nc.tensor.transpose(
                    n_ps[:, i : i + 1],
                    nr[0:1, (i % 4) * 128 : (i % 4 + 1) * 128],
                    ident[0:1, 0:1],
                )
            n_all = batch.tile([128, NT], F32, tag="n_all")
            nc.vector.tensor_copy(_rr(n_all), n_ps)

            ac_ps = ps_ac.tile([1, 256], F32, tag="ac")
            for i in range(NT):
                nc.tensor.matmul(
                    ac_ps,
                    lhsT=_rr(n_all[:, i : i + 1]),
                    rhs=_rr(Cr[:, i, :]),
                    start=(i == 0),
                    stop=(i == NT - 1),
                )
            s_ps = ps_s.tile([1, NT], F32, tag="s")
            nc.tensor.matmul(s_ps, lhsT=_rr(ones2_r[:, 0:1]), rhs=_rr(n_all))
            s_tot = batch.tile([1, 1], F32, tag="s_tot")
            nc.vector.reduce_sum(out=s_tot, in_=s_ps, axis=AX.X)
            r_s = batch.tile([1, 1], F32, tag="r_s")
            nc.vector.reciprocal(r_s, s_tot)
            ac_row = batch.tile([1, 256], F32, tag="ac_row")
            nc.scalar.activation(ac_row, ac_ps, AF.Copy, scale=r_s)

            acb = batch.tile([128, 256], F32, tag="acb")
            nc.gpsimd.partition_broadcast(acb, ac_row, channels=128)

            for i in range(NT):
                nc.vector.tensor_mul(o4_all[:, i, :], C[:, i, :], acb)
            out_r = out_ext[b].rearrange("(i p) d -> p i d", p=128)
            h = NT // 2
            nc.sync.dma_start(out=out_r[:, 0:h, 256:768], in_=aqo3[:, 0:h, :])
            nc.sync.dma_start(out=out_r[:, h:NT, 256:768], in_=aqo3[:, h:NT, :])
            nc.sync.dma_start(out=out_r[:, 0:h, 768:1024], in_=o4_all[:, 0:h, :])
            nc.sync.dma_start(out=out_r[:, h:NT, 768:1024], in_=o4_all[:, h:NT, :])

        prev = None
        for b in range(BPC):
            # ---- loads ----
            Q = batch.tile([128, D], F32, tag="Q")
            nc.sync.dma_start(out=Q, in_=qry_in[b])
            C = big.tile([128, NT, D], F32, tag="C")
            ctx_r = ctx_in[b].rearrange("(i p) d -> p i d", p=128)
            nc.sync.dma_start(out=C[:, 0 : NT // 2, :], in_=ctx_r[:, 0 : NT // 2, :])
            nc.sync.dma_start(out=C[:, NT // 2 : NT, :], in_=ctx_r[:, NT // 2 : NT, :])
            # out component 1 = context passthrough, straight from the C tile
            nc.sync.dma_start(
                out=out_ext[b].rearrange("(i p) d -> p i d", p=128)[:, :, 0:256],
                in_=C,
            )
            # rounded copy of Q for the fp32r AQ matmul
            Qr = batch.tile([128, D], F32, tag="Qr")
            nc.scalar.copy(_rr(Qr), Q)
            # rounded copy of C for the fp32r beta-weighted sum (split ACT/DVE)
            Cr = big.tile([128, NT, D], F32, tag="Cr")
            nc.scalar.copy(_rr(Cr[:, 0 : NT // 2, :]), C[:, 0 : NT // 2, :])
            nc.vector.tensor_copy(_rr(Cr[:, NT // 2 : NT, :]), C[:, NT // 2 : NT, :])

            # ---- Q^T, with w_m folded in: qwT[d, j] = Q[j, d] * w_m[d] ----
            qt_ps = ps_ct.tile([128, D], F32, tag="ct")
            for k in range(ND):
                nc.tensor.transpose(
                    qt_ps[:, k * 128 : (k + 1) * 128], Q[:, k * 128 : (k + 1) * 128], ident
                )
            qwT = batch.tile([128, D], F32, tag="qwT")
            for k in range(ND):
                nc.scalar.activation(
                    _rr(qwT[:, k * 128 : (k + 1) * 128]),
                    qt_ps[:, k * 128 : (k + 1) * 128],
                    AF.Copy,
                    scale=wm_cols[:, k : k + 1],
                )

            # ---- s_q column: rowsum(Q * w_q) ----
            sq_scr = batch.tile([128, D], F32, tag="sq_scr")
            sq_col = batch.tile([128, 1], F32, tag="sq_col")
            nc.vector.tensor_mul(sq_scr, Q, wq_b)
            nc.vector.reduce_sum(out=sq_col, in_=sq_scr, axis=AX.X)

            # ---- finish PREVIOUS batch early (its inputs are all ready) ----
            if prev is not None:
                beta_tail(prev)

            # ---- C^T via PE transposes ----
            ctT = big.tile([128, ND, NT, 128], F32, tag="ctT")
            for i2 in range(NT // 2):
                ct_ps = ps_ct.tile([128, 2 * ND * 128], F32, tag="ct")
                for u in range(2):
                    i = 2 * i2 + u
                    for k in range(ND):
                        nc.tensor.transpose(
                            ct_ps[:, (2 * u + k) * 128 : (2 * u + k + 1) * 128],
                            C[:, i, k * 128 : (k + 1) * 128],
                            ident,
                        )
                dst = _rr(ctT[:, :, 2 * i2 : 2 * i2 + 2, :])
                srcv = ct_ps.rearrange("p (t k x) -> p k t x", t=2, k=ND)
                if i2 % 2 == 0:
                    nc.scalar.copy(dst, srcv)
                else:
                    nc.vector.tensor_copy(dst, srcv)

            # ---- g^T = (Q*w_m) @ C^T and s_c^T = w_c^T @ C^T per 512-chunk ----
            ET = big.tile([128, T], F32, tag="ET")
            expsc_row = batch.tile([1, T], F32, tag="expsc_row")
            for c in range(T // 512):
                g_ps = ps_g.tile([128, 512], F32, tag="g")
                scr_ps = ps_u.tile([1, 512], F32, tag="u")
                for k in range(ND):
                    nc.tensor.matmul(
                        g_ps,
                        lhsT=_rr(qwT[:, k * 128 : (k + 1) * 128]),
                        rhs=_rr(ctT[:, k, 4 * c : 4 * (c + 1), :]),
                        start=(k == 0),
                        stop=(k == ND - 1),
                    )
                for k in range(ND):
                    nc.tensor.matmul(
                        scr_ps,
                        lhsT=_rr(wc_cols[:, k : k + 1]),
                        rhs=_rr(ctT[:, k, 4 * c : 4 * (c + 1), :]),
                        start=(k == 0),
                        stop=(k == ND - 1),
                    )
                nc.scalar.activation(
                    _rr(ET[:, c * 512 : (c + 1) * 512]), g_ps, AF.Exp, bias=sq_col
                )
                nc.scalar.activation(
                    expsc_row[:, c * 512 : (c + 1) * 512], scr_ps, AF.Exp
                )

            # ---- rowmax over j via GpSimd cross-partition max on E^T ----
            me_all = big.tile([128, T], F32, tag="me")
            n_rows = []
            for c in range(T // 512):
                nc.gpsimd.partition_all_reduce(
                    me_all[:, c * 512 : (c + 1) * 512],
                    ET[:, c * 512 : (c + 1) * 512],
                    channels=128,
                    reduce_op=bass_isa.ReduceOp.max,
                )
                nr = batch.tile([1, 512], F32, tag=f"n_row{c}")
                nc.vector.tensor_mul(
                    nr,
                    me_all[0:1, c * 512 : (c + 1) * 512],
                    expsc_row[:, c * 512 : (c + 1) * 512],
                )
                n_rows.append(nr)

            # ---- per t-tile: U = E @ [Q|1], AQ, batch staging writes ----
            aqo3 = big.tile([128, NT, 2 * D], F32, tag="aqo3")
            o4_all = big.tile([128, NT, D], F32, tag="o4")
            for i in range(NT):
                et_sl = ET[:, i * 128 : (i + 1) * 128]

                u_ps = ps_u.tile([128, 258], F32, tag="u")
                nc.tensor.matmul(u_ps[:, 0:256], lhsT=_rr(et_sl), rhs=_rr(Qr))
                nc.tensor.matmul(u_ps[:, 256:258], lhsT=_rr(et_sl), rhs=_rr(ones2_r))

                r_col = work.tile([128, 1], F32, tag="r_col")
                nc.vector.reciprocal(r_col, u_ps[:, 256:257])
                if i % 2 == 0:
                    nc.scalar.activation(
                        aqo3[:, i, 0:256], u_ps[:, 0:256], AF.Copy, scale=r_col
                    )
                else:
                    r_b = bass.AP(
                        tensor=r_col.tensor, offset=r_col.offset,
                        ap=[list(r_col.ap)[0], [0, 256]],
                    )
                    nc.vector.tensor_mul(aqo3[:, i, 0:256], u_ps[:, 0:256], r_b)
                nc.vector.tensor_mul(aqo3[:, i, 256:512], aqo3[:, i, 0:256], C[:, i, :])

            prev = (b, C, Cr, n_rows, aqo3, o4_all)

        beta_tail(prev)


_NC_CACHE = {}


def kernel(context, query, w, b, _trace=False):
    context = np.ascontiguousarray(context, dtype=np.float32)
    query = np.ascontiguousarray(query, dtype=np.float32)
    w = np.ascontiguousarray(w, dtype=np.float32)

    if "nc" not in _NC_CACHE:
        _NC_CACHE["nc"] = build_nc()
    nc = _NC_CACHE["nc"]

    in_maps = [
        {
            "context": context[i * BPC : (i + 1) * BPC],
            "query": query[i * BPC : (i + 1) * BPC],
            "w": w,
        }
        for i in range(NCORES)
    ]
    try:
        res = run_bass_kernel_spmd(
            nc, in_maps, core_ids=list(range(NCORES)), trace=_trace
        )
    except Exception:
        # A previous process may have left the device wedged; reset and retry.
        import ctypes

        import jax

        jax.devices()
        lib = ctypes.CDLL("/opt/axon/libaxon_pjrt.so")
        if hasattr(lib, "axon_reset"):
            lib.axon_reset()
        res = run_bass_kernel_spmd(
            nc, in_maps, core_ids=list(range(NCORES)), trace=_trace
        )
    out = np.concatenate([res.results[i]["out"] for i in range(NCORES)], axis=0)
    if _trace:
        kernel.last_exec_time_ns = res.exec_time_ns
        kernel.last_results = res
    return out


if __name__ == "__main__":
    rng = np.random.default_rng(0)
    inputs = {
        "context": rng.standard_normal((B, T, D), dtype=np.float32),
        "query": rng.standard_normal((B, J, D), dtype=np.float32),
        "w": (rng.standard_normal(3 * D).astype(np.float32) / np.sqrt(3 * D)),
        "b": np.zeros(1, np.float32),
    }
    out = kernel(**inputs)
    print("out", out.shape, out.dtype, float(np.abs(out).mean()))



# revision 49
# speedup vs baseline: 1.1420x; 1.1420x over previous
"""AttentionFlow Trainium2 Bass kernel (bf16 pipeline, DMA-first schedule).

Math (per batch):
  d = 256; w = [w_c | w_q | w_m]
  sim[t,j] = s_c[t] + s_q[j] + sum_d C[t,d] w_m[d] Q[j,d]   (+b, b==0 path)
  attn = softmax_j(sim);  AQ = attn @ Q
  beta = softmax_t(max_j sim);  AC = beta @ C
  out = concat([C, AQ, C*AQ, C*AC], axis=-1)

Key implementation choices vs the fp32r baseline (94us):
  - s_c is folded INTO the scores pre-exp (rank-1 K=1 matmul adds s_c[t]
    along partitions): E' = exp(g + s_q + s_c).  The exp(s_c) factor
    cancels exactly in the attn softmax division, and rowmax_j(E') IS the
    beta numerator directly -- the whole single-partition exp/mul row
    pipeline of the baseline disappears.
  - rowmax over j (partition dim of E'^T) via PE transposes of E'^T tiles
    + one DVE reduce_max, instead of gpsimd partition_all_reduce (2us per
    call, serialized the pipeline).
  - AC broadcast over partitions via a K=1 PE matmul (ones x ac_row)
    instead of gpsimd partition_broadcast.
  - All matmuls/transposes in bf16 (tolerance is 2e-2 absmax-rel; bf16
    keeps us ~1e-3): transposes 1 cyc/row (vs 2 fp32), no fp32r
    rounded-producer hassles, DVE copies at 2x packed rate, SBUF halved.
  - DMA-first program order: all input loads + context-passthrough output
    writes issue before any compute so HBM is saturated from t~0 (the
    baseline idled DMA for the first ~9us and between batches).
  - o4 elementwise mul on gpsimd (otherwise idle), o3/aq split DVE/ACT;
    wide multi-dim APs everywhere to amortize the per-instruction fixed
    cost (DVE ~120c, ACT ~172-224c).
Output traffic (16 MiB/core write + 4.5 MiB read at ~358 GB/s) is the
~60us roofline; everything else hides under it.
"""

import numpy as np

import concourse.bass as bass
import concourse.mybir as mybir
import concourse.tile as tile
from concourse import bacc
from concourse.bass_utils import run_bass_kernel_spmd
from concourse.masks import make_identity

F32 = mybir.dt.float32
BF16 = mybir.dt.bfloat16
AF = mybir.ActivationFunctionType
ALU = mybir.AluOpType
AX = mybir.AxisListType

B, T, J, D = 32, 1024, 128, 256
NCORES = 8
BPC = B // NCORES      # batches per core
NT = T // 128          # t-tiles per batch
ND = D // 128          # d-tiles


def _bcast_row(ap_1d, nparts):
    """DRAM AP [n] -> [nparts, n] with partition stride 0 (DMA broadcast)."""
    return bass.AP(
        tensor=ap_1d.tensor, offset=ap_1d.offset, ap=[[0, nparts]] + list(ap_1d.ap)
    )


def build_nc():
    nc = bacc.Bacc()
    ctx_in = nc.declare_dram_parameter("context", [BPC, T, D], F32, isOutput=False)
    qry_in = nc.declare_dram_parameter("query", [BPC, J, D], F32, isOutput=False)
    w_in = nc.declare_dram_parameter("w", [3 * D], F32, isOutput=False)
    out_ext = nc.declare_dram_parameter("out", [BPC, T, 4 * D], F32, isOutput=True)

    with tile.TileContext(nc) as tc:
        _body(tc, ctx_in, qry_in, w_in, out_ext)
    nc.finalize()
    return nc


def _body(tc, ctx_in, qry_in, w_in, out_ext):
    nc = tc.nc
    from contextlib import ExitStack

    with ExitStack() as ctx:
        consts = ctx.enter_context(tc.tile_pool(name="consts", bufs=1))
        cq = ctx.enter_context(tc.tile_pool(name="cq", bufs=BPC))
        big = ctx.enter_context(tc.tile_pool(name="big", bufs=3))
        work = ctx.enter_context(tc.tile_pool(name="work", bufs=4))
        out_p = ctx.enter_context(tc.tile_pool(name="out_p", bufs=3))
        # PSUM budget (8 banks): t 2 + g 2 + u 2 + m 2
        ps_t = ctx.enter_context(tc.tile_pool(name="ps_t", bufs=2, space="PSUM"))
        ps_g = ctx.enter_context(tc.tile_pool(name="ps_g", bufs=2, space="PSUM"))
        ps_u = ctx.enter_context(tc.tile_pool(name="ps_u", bufs=2, space="PSUM"))
        ps_m = ctx.enter_context(tc.tile_pool(name="ps_m", bufs=2, space="PSUM"))

        # ---- input DMAs first: saturate HBM from t=0.  Each dma_start costs
        # ~0.8us of issue time on its engine, so: few, big, C0/Q0 first,
        # passthrough writes on gpsimd's queue. ----
        C_tiles, Q_tiles = [], []
        wc_raw = consts.tile([128, ND], F32)
        wm_cols = consts.tile([128, ND], F32)
        wq_b = consts.tile([128, D], F32)
        # Q0 + w first (small, fast): the whole Q-side of batch 0 (Qr1, qt,
        # qwT, sq) computes while C0's 1 MiB is still in flight.
        def load_C(b):
            C = cq.tile([128, NT, D], F32, tag="C")
            nc.sync.dma_start(
                out=C, in_=ctx_in[b].rearrange("(i p) d -> p i d", p=128)
            )
            C_tiles.append(C)

        def load_Q(b):
            Q = cq.tile([128, D], F32, tag="Q")
            nc.sync.dma_start(out=Q, in_=qry_in[b])
            Q_tiles.append(Q)

        load_C(0)
        load_Q(0)
        nc.sync.dma_start(
            out=wc_raw, in_=w_in[0 : 2 * 128].rearrange("(o p) -> p o", p=128)
        )
        nc.sync.dma_start(
            out=wm_cols,
            in_=w_in[2 * D : 2 * D + 2 * 128].rearrange("(o p) -> p o", p=128),
        )
        load_C(1)
        nc.sync.dma_start(out=wq_b, in_=_bcast_row(w_in[D : 2 * D], 128))
        load_C(2)
        load_Q(1)
        load_C(3)
        load_Q(2)
        load_Q(3)
        # context passthrough: out[:, :, 0:256] straight from the C tiles
        for b in range(BPC):
            nc.gpsimd.dma_start(
                out=out_ext[b].rearrange("(i p) d -> p i d", p=128)[:, :, 0:256],
                in_=C_tiles[b],
            )

        # ---- constants ----
        ident_b = consts.tile([128, 128], BF16)
        make_identity(nc, ident_b)
        ones_row_b = consts.tile([1, 128], BF16)
        nc.vector.memset(ones_row_b, 1.0)
        ones_col_b = consts.tile([128, 1], BF16)
        nc.vector.memset(ones_col_b, 1.0)

        # HAM warm-up: the PE clock gate sits at 1.2 GHz until ~3.4us of
        # sustained activity.  Fill the initial DMA-wait dead zone with
        # back-to-back dummy transposes so batch 0's real matmuls start at
        # 2.4 GHz.  (One throwaway read releases the PSUM slot.)
        warm_ps = ps_m.tile([128, 128], BF16, tag="m")
        for _ in range(36):
            nc.tensor.transpose(warm_ps, ident_b, ident_b)
        warm_rd = work.tile([128, 1], BF16, tag="warm_rd")
        nc.vector.reduce_max(out=warm_rd, in_=warm_ps, axis=AX.X)

        for b in range(BPC):
            C = C_tiles[b]
            Q = Q_tiles[b]

            # ---- Qr1 = [Q | 1 1] in bf16 (one U matmul per tile) ----
            Qr1 = work.tile([128, D + 2], BF16, tag="Qr1")
            nc.vector.memset(Qr1[:, D : D + 2], 1.0)
            nc.vector.tensor_copy(Qr1[:, 0:D], Q)

            # ---- s_q column: rowsum(Q * w_q) on gpsimd (off critical path) ----
            sq_scr = work.tile([128, D], F32, tag="sq_scr")
            sq_col = work.tile([128, 1], F32, tag="sq_col")
            nc.vector.tensor_mul(sq_scr, Q, wq_b)
            nc.vector.reduce_sum(out=sq_col, in_=sq_scr, axis=AX.X)

            # ---- qwT[d, j] = Q[j, d] * w_m[d] + w_c[d] via bf16 transposes +
            # one ACT scale+bias pass.  The +w_c bias makes the g matmul
            # compute g[j,t] + s_c[t] directly (sum_d wc[d] C^T[d,t] = s_c):
            # the entire separate s_c pipeline disappears. ----
            qt_ps = ps_t.tile([128, D], BF16, tag="t")
            for k in range(ND):
                nc.tensor.transpose(
                    qt_ps[:, k * 128 : (k + 1) * 128],
                    Qr1[:, k * 128 : (k + 1) * 128],
                    ident_b,
                )
            qwT = work.tile([128, D], BF16, tag="qwT")
            for k in range(ND):
                nc.scalar.activation(
                    qwT[:, k * 128 : (k + 1) * 128],
                    qt_ps[:, k * 128 : (k + 1) * 128],
                    AF.Identity,
                    scale=wm_cols[:, k : k + 1],
                    bias=wc_raw[:, k : k + 1],
                )

            # ---- Cb = bf16(C), halves split DVE/ACT ----
            Cb = big.tile([128, NT, D], BF16, tag="Cb")
            nc.vector.tensor_copy(Cb[:, 0 : NT // 2, :], C[:, 0 : NT // 2, :])
            nc.scalar.copy(Cb[:, NT // 2 : NT, :], C[:, NT // 2 : NT, :])

            # ---- C^T via bf16 PE transposes ----
            ctT = big.tile([128, ND, NT, 128], BF16, tag="ctT")
            for h in range(2):
                ct_ps = ps_t.tile([128, 8 * 128], BF16, tag="t")
                for u in range(4):
                    i = 4 * h + u
                    for k in range(ND):
                        nc.tensor.transpose(
                            ct_ps[:, (2 * u + k) * 128 : (2 * u + k + 1) * 128],
                            Cb[:, i, k * 128 : (k + 1) * 128],
                            ident_b,
                        )
                srcv = ct_ps.rearrange("p (t k x) -> p k t x", t=4, k=ND)
                if h == 0:
                    nc.vector.tensor_copy(ctT[:, :, 0:4, :], srcv)
                else:
                    nc.scalar.copy(ctT[:, :, 4:8, :], srcv)

            # ---- E'^T = exp(g + s_q + s_c) per 512-chunk; s_c added as a
            #      rank-1 (K=1) matmul so it lands along the free dim ----
            ET = big.tile([128, T], BF16, tag="ET")
            for c in range(2):
                g_ps = ps_g.tile([128, 512], F32, tag="g")
                for k in range(ND):
                    nc.tensor.matmul(
                        g_ps,
                        lhsT=qwT[:, k * 128 : (k + 1) * 128],
                        rhs=ctT[:, k, 4 * c : 4 * (c + 1), :],
                        start=(k == 0),
                        stop=(k == ND - 1),
                    )
                nc.scalar.activation(
                    ET[:, c * 512 : (c + 1) * 512], g_ps, AF.Exp, bias=sq_col
                )

            # ---- n[t] = max_j E'[t,:] via PE transposes + DVE reduce_max ----
            et_ps = ps_t.tile([128, 8 * 128], BF16, tag="t")
            for i in range(NT):
                nc.tensor.transpose(
                    et_ps[:, i * 128 : (i + 1) * 128],
                    ET[:, i * 128 : (i + 1) * 128],
                    ident_b,
                )
            n_sb = work.tile([128, NT], BF16, tag="n_sb")
            nc.vector.reduce_max(
                out=n_sb,
                in_=et_ps.rearrange("p (i x) -> p i x", i=NT),
                axis=AX.X,
            )

            # ---- beta path: AC = (n @ C) / sum(n); broadcast via K=1 matmul ----
            ac_ps = ps_m.tile([1, 256], F32, tag="m")
            for i in range(NT):
                nc.tensor.matmul(
                    ac_ps,
                    lhsT=n_sb[:, i : i + 1],
                    rhs=Cb[:, i, :],
                    start=(i == 0),
                    stop=(i == NT - 1),
                )
            s_ps = ps_m.tile([1, NT], F32, tag="m")
            nc.tensor.matmul(s_ps, lhsT=ones_col_b, rhs=n_sb)
            s_tot = work.tile([1, 1], F32, tag="s_tot")
            nc.vector.reduce_sum(out=s_tot, in_=s_ps, axis=AX.X)
            r_s = work.tile([1, 1], F32, tag="r_s")
            nc.vector.reciprocal(r_s, s_tot)
            ac_row = work.tile([1, 256], BF16, tag="ac_row")
            nc.scalar.activation(ac_row, ac_ps, AF.Copy, scale=r_s)
            acb = work.tile([128, 256], F32, tag="acb")
            acb_ps = ps_m.tile([128, 256], F32, tag="m")
            nc.tensor.matmul(acb_ps, lhsT=ones_row_b, rhs=ac_row)
            nc.scalar.copy(acb, acb_ps)

            # ---- o4 = C * AC on gpsimd (broadcast acb over t-tiles) ----
            # ---- o4 = C * AC into the merged staging tile (cols 512:768) ----
            out_r = out_ext[b].rearrange("(i p) d -> p i d", p=128)
            h = NT // 2
            aqo3 = out_p.tile([128, NT, 3 * D], F32, tag="aqo3")
            acb_bc = bass.AP(
                tensor=acb.tensor, offset=acb.offset,
                ap=[list(acb.ap)[0], [0, h]] + list(acb.ap)[1:],
            )
            nc.vector.tensor_mul(aqo3[:, 0:h, 512:768], C[:, 0:h, :], acb_bc)
            nc.vector.tensor_mul(aqo3[:, h:NT, 512:768], C[:, h:NT, :], acb_bc)

            # ---- U = E' @ [Q|1]; AQ = U[:,0:256]/U[:,256]; o3 = AQ*C ----
            for i in range(NT):
                et_sl = ET[:, i * 128 : (i + 1) * 128]
                u_ps = ps_u.tile([128, D + 2], F32, tag="u")
                nc.tensor.matmul(u_ps, lhsT=et_sl, rhs=Qr1)
                r_col = work.tile([128, 1], F32, tag="r_col")
                nc.vector.reciprocal(r_col, u_ps[:, D : D + 1])
                nc.scalar.activation(
                    aqo3[:, i, 0:256], u_ps[:, 0:256], AF.Copy, scale=r_col
                )
                last = b == BPC - 1
                step = 2 if last else NT // 2
                if (i + 1) % step == 0:
                    lo, hi = i + 1 - step, i + 1
                    nc.vector.tensor_mul(
                        aqo3[:, lo:hi, 256:512],
                        aqo3[:, lo:hi, 0:256],
                        C[:, lo:hi, :],
                    )
                    nc.sync.dma_start(
                        out=out_r[:, lo:hi, 256:1024], in_=aqo3[:, lo:hi, :]
                    )


_NC_CACHE = {}


def kernel(context, query, w, b, _trace=False):
    context = np.ascontiguousarray(context, dtype=np.float32)
    query = np.ascontiguousarray(query, dtype=np.float32)
    w = np.ascontiguousarray(w, dtype=np.float32)

    if "nc" not in _NC_CACHE:
        _NC_CACHE["nc"] = build_nc()
    nc = _NC_CACHE["nc"]

    in_maps = [
        {
            "context": context[i * BPC : (i + 1) * BPC],
            "query": query[i * BPC : (i + 1) * BPC],
            "w": w,
        }
        for i in range(NCORES)
    ]
    try:
        res = run_bass_kernel_spmd(
            nc, in_maps, core_ids=list(range(NCORES)), trace=_trace
        )
    except Exception:
        # A previous process may have left the device wedged; reset and retry.
        import ctypes

        import jax

        jax.devices()
        lib = ctypes.CDLL("/opt/axon/libaxon_pjrt.so")
        if hasattr(lib, "axon_reset"):
            lib.axon_reset()
        res = run_bass_kernel_spmd(
            nc, in_maps, core_ids=list(range(NCORES)), trace=_trace
        )
    out = np.concatenate([res.results[i]["out"] for i in range(NCORES)], axis=0)
    if _trace:
        kernel.last_exec_time_ns = res.exec_time_ns
        kernel.last_results = res
    return out


if __name__ == "__main__":
    rng = np.random.default_rng(0)
    inputs = {
        "context": rng.standard_normal((B, T, D), dtype=np.float32),
        "query": rng.standard_normal((B, J, D), dtype=np.float32),
        "w": (rng.standard_normal(3 * D).astype(np.float32) / np.sqrt(3 * D)),
        "b": np.zeros(1, np.float32),
    }
    out = kernel(**inputs)
    print("out", out.shape, out.dtype, float(np.abs(out).mean()))
